# revision 1
# baseline (speedup 1.0000x reference)
"""Conv2Demod (StyleGAN modulated conv) Trainium2 kernel.

Full-input contract: kernel(**inputs) takes the unsharded tensors from
setup_inputs() and returns the full [16, 512, 64, 64] output.

Strategy: data-parallel over batch across 8 NeuronCores (2 samples/core).
Per core the kernel computes, fully on-device:
  styles  = ws @ A_w.T + A_b                      (PE, N=2 matmuls)
  wmod    = weight * styles[ci]                   (DVE per-partition scale -> bf16)
  sumsq   = sum_{ci,kh,kw} (weight*styles)^2      (ACT square + PE matmul w/ styles^2)
  dcoef   = 1/sqrt(sumsq + eps)                   (ACT sqrt + DVE reciprocal)
  conv    = 3x3 grouped conv as 36 shift-matmuls  (PE, bf16, fp32 PSUM accum)
  out     = dcoef * (conv + (B/dcoef) * noise)    (K=1 matmul into PSUM + DVE scale)

Host side does layout only: SAME-padding into a 66x66 bf16 image, weight
transpose to [tap, ci, co], A_w / ws transposes, and the final gather.
"""

import numpy as np
import ml_dtypes

import concourse.bass as bass
import concourse.tile as tile
from concourse import bacc, mybir
from concourse.bass import ts
from concourse.bass_utils import run_bass_kernel_spmd
from concourse.masks import make_identity

N_CORES = 8
B_SZ, C, Z, K, H, W = 16, 512, 512, 3, 64, 64
S = B_SZ // N_CORES            # samples per core
HP, WP = H + 2, W + 2          # padded spatial
HW = H * W
P = 128                        # partitions
NT = C // P                    # channel tiles
TAPS = K * K
EPS = 1e-8
FD = 512                       # matmul moving free dim == one PSUM bank of f32
NCHUNK = HW // FD              # spatial chunks per (sample, co-tile)
CPG = 4                        # chunks in flight per PSUM group
GROUPS = NCHUNK // CPG
ROWS_PER_CHUNK = FD // W       # output rows covered by one chunk

BF16 = mybir.dt.bfloat16
F32 = mybir.dt.float32

LAST_RESULT = None             # test harness peeks at this for profiling info
_NC_CACHE = {}


def _build_nc(with_noise: bool):
    nc = bacc.Bacc(None)

    xpad = nc.dram_tensor("xpad", [S, C, HP, WP], BF16, kind="ExternalInput")
    wT = nc.dram_tensor("wT", [TAPS, C, C], F32, kind="ExternalInput")
    awT = nc.dram_tensor("awT", [Z, C], F32, kind="ExternalInput")
    wsT = nc.dram_tensor("wsT", [Z, S], F32, kind="ExternalInput")
    ab = nc.dram_tensor("ab", [C], F32, kind="ExternalInput")
    if with_noise:
        bp = nc.dram_tensor("bp", [C], F32, kind="ExternalInput")
        noise = nc.dram_tensor("noise", [S, HW], F32, kind="ExternalInput")
    out = nc.dram_tensor("out", [S, C, HW], F32, kind="ExternalOutput")

    ab_r = ab.rearrange("(t p u) -> t p u", p=P, u=1)
    awT_r = awT.rearrange("(t p) c -> t p c", p=P)
    wsT_r = wsT.rearrange("(t p) s -> t p s", p=P)
    if with_noise:
        bp_r = bp.rearrange("(t p u) -> t p u", p=P, u=1)
        noise_r = noise.rearrange("s (u m) -> s u m", u=1)

    with tile.TileContext(nc) as tc:
        with (
            tc.tile_pool(name="persist", bufs=1) as persist,
            tc.tile_pool(name="wstream", bufs=8) as wstream,
            tc.tile_pool(name="sqp", bufs=2) as sqp,
            tc.tile_pool(name="outp", bufs=4) as outp,
            tc.tile_pool(name="psum", bufs=8, space="PSUM") as psum,
        ):
            # ---- phase A: small constants first (styles inputs), then s0 image ----
            awT_sb = [wstream.tile([P, C], F32, tag="wraw", name=f"awT{z}") for z in range(NT)]
            wsT_sb = [persist.tile([P, S], F32, tag=f"wsT{z}", name=f"wsT{z}") for z in range(NT)]
            ab_sb = [persist.tile([P, 1], F32, tag=f"ab{t}", name=f"ab{t}") for t in range(NT)]
            for t in range(NT):
                nc.sync.dma_start(out=wsT_sb[t], in_=wsT_r[t])
                nc.sync.dma_start(out=ab_sb[t], in_=ab_r[t])
                nc.sync.dma_start(out=awT_sb[t], in_=awT_r[t])

            if with_noise:
                bp_sb = [persist.tile([P, 1], F32, tag=f"bp{t}", name=f"bp{t}") for t in range(NT)]
                for t in range(NT):
                    nc.sync.dma_start(out=bp_sb[t], in_=bp_r[t])
                noise_b = [persist.tile([1, HW], BF16, tag=f"nsb{s}", name=f"nsb{s}") for s in range(S)]
                for s in range(S):
                    for ch in range(NCHUNK):
                        nstg = sqp.tile([1, FD], F32, tag="nstg", name="nstg", bufs=2)
                        nc.sync.dma_start(out=nstg, in_=noise_r[s][0:1, ts(ch, FD)])
                        nc.vector.tensor_copy(
                            out=noise_b[s][0:1, ts(ch, FD)], in_=nstg[:]
                        )
                ident = persist.tile([P, P], F32, tag="ident")
                make_identity(nc, ident[:])

            xp_sb = [
                [persist.tile([P, HP, WP], BF16, tag=f"xp{s}_{ci}", name=f"xp{s}_{ci}") for ci in range(NT)]
                for s in range(S)
            ]
            for ci in range(NT):
                for q in range(4):
                    nc.sync.dma_start(
                        out=xp_sb[0][ci][q * 32 : (q + 1) * 32, :, :],
                        in_=xpad[0, ci * P + q * 32 : ci * P + (q + 1) * 32, :, :],
                    )

            # styles[ci, s] = sum_z A_w[ci, z] * ws[s, z] + A_b[ci]
            styles = [persist.tile([P, S], F32, tag=f"st{t}", name=f"st{t}") for t in range(NT)]
            styles_sq = [persist.tile([P, S], BF16, tag=f"stq{t}", name=f"stq{t}") for t in range(NT)]
            for cb in range(NT):
                ps = psum.tile([P, FD], F32, tag="ps")
                for zt in range(NT):
                    nc.tensor.matmul(
                        ps[:, 0:S],
                        lhsT=awT_sb[zt][:, ts(cb, P)],
                        rhs=wsT_sb[zt][:, 0:S],
                        start=(zt == 0),
                        stop=(zt == NT - 1),
                    )
                nc.vector.tensor_scalar_add(styles[cb][:], ps[:, 0:S], ab_sb[cb][:])
                nc.vector.tensor_mul(styles_sq[cb][:], styles[cb][:], styles[cb][:])

            # ---- phase B: weight stream -> modulated bf16 weights + sumsq ----
            psum_d = [psum.tile([P, FD], F32, tag="ps", name="psd") for _ in range(NT)]
            wmod = [
                [[None] * NT for _ in range(TAPS)] for _ in range(S)
            ]
            for tap in range(TAPS):
                for ci in range(NT):
                    wr = wstream.tile([P, C], F32, tag="wraw")
                    for q in range(4):
                        nc.sync.dma_start(
                            out=wr[q * 32 : (q + 1) * 32, :],
                            in_=wT[tap, ci * P + q * 32 : ci * P + (q + 1) * 32, :],
                        )
                    for s in range(S):
                        wm = persist.tile([P, C], BF16, tag=f"wm{s}_{tap}_{ci}")
                        nc.vector.tensor_scalar_mul(
                            wm[:], wr[:], styles[ci][:, s : s + 1]
                        )
                        wmod[s][tap][ci] = wm
                    sqt = sqp.tile([P, C], BF16, tag="sq")
                    nc.scalar.square(sqt[:], wr[:])
                    for cb in range(NT):
                        nc.tensor.matmul(
                            psum_d[cb][:, 0:S],
                            lhsT=sqt[:, ts(cb, P)],
                            rhs=styles_sq[ci][:, 0:S],
                            start=(tap == 0 and ci == 0),
                            stop=(tap == TAPS - 1 and ci == NT - 1),
                            skip_group_check=True,
                        )

            for s in range(1, S):
                for ci in range(NT):
                    nc.sync.dma_start(out=xp_sb[s][ci], in_=xpad[s, ts(ci, P), :, :])

            # ---- phase C: dcoef + noise-bias row ----
            dcoef = [persist.tile([P, S], F32, tag=f"dc{t}", name=f"dc{t}") for t in range(NT)]
            eps_sb = persist.tile([P, 1], F32, tag="eps")
            nc.vector.memset(eps_sb[:], EPS)
            if with_noise:
                b_rows = [persist.tile([1, C], BF16, tag=f"brow{s}", name=f"brow{s}") for s in range(S)]
            for cb in range(NT):
                sq_coef = persist.tile([P, S], F32, tag=f"sqc{cb}")
                nc.scalar.activation(
                    sq_coef[:],
                    psum_d[cb][:, 0:S],
                    mybir.ActivationFunctionType.Sqrt,
                    bias=eps_sb[:],
                )
                nc.vector.reciprocal(dcoef[cb][:], sq_coef[:])
                if with_noise:
                    bmod = persist.tile([P, S], F32, tag=f"bm{cb}")
                    nc.vector.tensor_scalar_mul(bmod[:], sq_coef[:], bp_sb[cb][:])
                    for s in range(S):
                        pt = psum.tile([P, FD], F32, tag="ps")
                        nc.tensor.transpose(pt[0:1, 0:P], bmod[:, s : s + 1], ident[:])
                        nc.vector.tensor_copy(
                            out=b_rows[s][0:1, ts(cb, P)], in_=pt[0:1, 0:P]
                        )

            # ---- phase D: the conv ----
            for s in range(S):
                for g in range(GROUPS):
                    for co in range(NT):
                        pss = [psum.tile([P, FD], F32, tag="ps", name="psc") for _ in range(CPG)]
                        for tap in range(TAPS):
                            kh, kw = divmod(tap, K)
                            for ci in range(NT):
                                lh = wmod[s][tap][ci][:, ts(co, P)]
                                for c in range(CPG):
                                    h0 = (g * CPG + c) * ROWS_PER_CHUNK
                                    rhs = xp_sb[s][ci][
                                        :, h0 + kh : h0 + kh + ROWS_PER_CHUNK, kw : kw + W
                                    ]
                                    nc.tensor.matmul(
                                        pss[c][:, :],
                                        lhsT=lh,
                                        rhs=rhs,
                                        start=(tap == 0 and ci == 0),
                                        stop=(
                                            not with_noise
                                            and tap == TAPS - 1
                                            and ci == NT - 1
                                        ),
                                        skip_group_check=True,
                                    )
                        for c in range(CPG):
                            chunk = g * CPG + c
                            if with_noise:
                                nc.tensor.matmul(
                                    pss[c][:, :],
                                    lhsT=b_rows[s][0:1, ts(co, P)],
                                    rhs=noise_b[s][0:1, ts(chunk, FD)],
                                    start=False,
                                    stop=True,
                                    skip_group_check=True,
                                )
                            ot = outp.tile([P, FD], F32, tag="ot")
                            nc.vector.tensor_scalar_mul(
                                ot[:], pss[c][:, :], dcoef[co][:, s : s + 1]
                            )
                            nc.sync.dma_start(
                                out=out[s, ts(co, P), ts(chunk, FD)], in_=ot
                            )

    nc.finalize()
    return nc


def kernel(img, ws, noise, weight, A_w, A_b, B_param):
    global _NC_CACHE, LAST_RESULT
    img = np.asarray(img, dtype=np.float32)
    ws = np.asarray(ws, dtype=np.float32)
    noise = np.asarray(noise, dtype=np.float32)
    weight = np.asarray(weight, dtype=np.float32)
    A_w = np.asarray(A_w, dtype=np.float32)
    A_b = np.asarray(A_b, dtype=np.float32)
    B_param = np.asarray(B_param, dtype=np.float32)

    # host-side layout prep (no math beyond dtype rounding)
    xpad = np.zeros((B_SZ, C, HP, WP), dtype=ml_dtypes.bfloat16)
    xpad[:, :, 1 : H + 1, 1 : W + 1] = img
    wT = np.ascontiguousarray(weight.transpose(2, 3, 1, 0)).reshape(TAPS, C, C)
    awT = np.ascontiguousarray(A_w.T)
    noise_flat = np.ascontiguousarray(noise.reshape(B_SZ, HW))

    with_noise = bool(np.any(B_param))
    if with_noise not in _NC_CACHE:
        _NC_CACHE[with_noise] = _build_nc(with_noise)
    nc = _NC_CACHE[with_noise]

    in_maps = []
    for c in range(N_CORES):
        sl = slice(c * S, (c + 1) * S)
        m = {
            "xpad": np.ascontiguousarray(xpad[sl]),
            "wT": wT,
            "awT": awT,
            "wsT": np.ascontiguousarray(ws[sl].T),
            "ab": A_b,
        }
        if with_noise:
            m["bp"] = B_param
            m["noise"] = noise_flat[sl]
        in_maps.append(m)

    res = run_bass_kernel_spmd(nc, in_maps, core_ids=list(range(N_CORES)))
    LAST_RESULT = res
    out = np.concatenate([res.results[c]["out"] for c in range(N_CORES)], axis=0)
    return out.reshape(B_SZ, C, H, W)

